# revision 1
# baseline (speedup 1.0000x reference)
"""Trainium2 Bass kernel for BitnetFeedForward (nn_BitnetFeedForward_14611478741500).

Math (per token row x, full weight W):
    bitlinear158(x, W) = xq @ Wq^T
      xn = x/||x|| * sqrt(d);  xq = round(xn*s)/s, s = 127/absmax(xn)
      Wq = clip(round(W*sw), -1, 1)/sw, sw = 1/mean|W|
    out = bitlinear158(gelu(gelu(bitlinear158(x, W1))), W2)

Key algebraic facts used:
  * The rms-norm is a positive per-token scalar, so it cancels inside the
    activation quant: round(xn * 127/absmax(xn)) == round(x * 127/absmax(x)).
    Only the dequant factor c = absmax(x)*sqrt(d)/(127*max(||x||,1e-12))
    remembers the norm.
  * Quantized activations are integers in [-127,127] (exact in bf16) and
    quantized weights are in {-1,0,1}; we feed the PE wq+192 (still exact in
    bf16, and the ternary round happens for free in the bf16 output
    conversion) and subtract 192*rowsum(xq) via the per-partition bias of
    the PSUM evacuation. All matmul arithmetic is integer-exact in fp32.

Distribution: pure data-parallel over the 8192 token rows (1024/core).
Weight-tensor mean|W| stats are sharded across cores and combined with a
single tiny (two scalars) AllReduce; everything else is core-local.
"""

import numpy as np
from contextlib import ExitStack

import concourse.bass as bass
import concourse.bass_isa as bass_isa
import concourse.mybir as mybir
from concourse import tile
from concourse import library_config

F32 = mybir.dt.float32
BF16 = mybir.dt.bfloat16
AX = mybir.AxisListType
OP = mybir.AluOpType
AF = mybir.ActivationFunctionType

P = 128
MAGIC = 12582912.0          # 1.5 * 2**23: fp32 round-to-nearest-even trick
WMAGIC = 192.0              # 1.5 * 2**7: bf16 RNE round for the ternary range
CLIP_B = 1.4990234375       # clamp bound making clamp+round == round+clip
EPS = 1e-5
CW = 2048                   # stats chunk width (junk tile width)

N_CORES = 8
FULL_T, DIM, INNER = 8192, 2048, 8192


def build_nc(T, D, I, n_cores=N_CORES, full_stats=False, debug=False, split_waits=True):
    """Emit the per-core SPMD program. T tokens/core, x:[T,D] -> out:[T,D]."""
    n_m = T // P           # token blocks
    n_k1 = D // P          # layer-1 contraction tiles
    n_nb1 = I // 512       # layer-1 output column blocks
    n_k2 = I // P          # layer-2 contraction tiles
    n_nb2 = D // 512       # layer-2 output column blocks
    d_shard = D if full_stats else D // n_cores
    i_shard = I if full_stats else I // n_cores

    nc = bass.Bass(debug=debug)

    x_d = nc.declare_dram_parameter("x", [T, D], F32, isOutput=False)
    w1t_d = nc.declare_dram_parameter("w1t", [D, I], F32, isOutput=False)
    w2t_d = nc.declare_dram_parameter("w2t", [I, D], F32, isOutput=False)
    # per-core stats shards (host slices of w1t/w2t rows)
    w1s_d = nc.declare_dram_parameter("w1s", [d_shard, I], F32, isOutput=False)
    w2s_d = nc.declare_dram_parameter("w2s", [i_shard, D], F32, isOutput=False)
    out_d = nc.declare_dram_parameter("out", [T, D], F32, isOutput=True)

    xq_d = nc.dram_tensor("xq_scr", [T, D], BF16)
    g2_d = nc.dram_tensor("g2_scr", [T, I], F32)
    hq_d = nc.dram_tensor("hq_scr", [T, I], BF16)
    if not full_stats:
        cc_in = nc.dram_tensor("cc_in", [1, 2], F32)
        cc_out = nc.dram_tensor("cc_out", [1, 2], F32, addr_space="Shared")

    HQC = 512              # h-quant chunk width
    n_hc = I // HQC

    with tile.TileContext(nc) as tc, ExitStack() as ctx:
        persist = ctx.enter_context(tc.tile_pool(name="persist", bufs=1))

        def pt(shape, dtype, tag):
            return persist.tile(shape, dtype, tag=tag, name=tag)

        junk = pt([P, CW], F32, "junk")
        ss_x = pt([P, n_m], F32, "ss_x")
        am_x = pt([P, n_m], F32, "am_x")
        rs_x = pt([P, n_m], F32, "rs_x")
        s_x = pt([P, n_m], F32, "s_x")
        cm1 = pt([P, n_m], F32, "cm1")
        bias1 = pt([P, n_m], F32, "bias1")
        ssh_p = pt([P, n_m * n_nb1], F32, "ssh_p")
        amh_p = pt([P, n_m * n_nb1], F32, "amh_p")
        rsh_p = pt([P, n_m * n_hc], F32, "rsh_p")
        ss_h = pt([P, n_m], F32, "ss_h")
        am_h = pt([P, n_m], F32, "am_h")
        rs_h = pt([P, n_m], F32, "rs_h")
        s_h = pt([P, n_m], F32, "s_h")
        cm2 = pt([P, n_m], F32, "cm2")
        bias2 = pt([P, n_m], F32, "bias2")
        wsum = pt([P, 2], F32, "wsum")   # [:,0]=sum|w1|, [:,1]=sum|w2|
        dw = pt([P, 2], F32, "dw")       # weight dequant = max(mean, eps)
        sw = pt([P, 2], F32, "sw")       # weight quant scale = 1/dequant
        t8a = pt([P, n_m], F32, "t8a")
        t8b = pt([P, n_m], F32, "t8b")
        t8c = pt([P, n_m], F32, "t8c")
        y0 = pt([P, n_m], F32, "y0")
        y1 = pt([P, n_m], F32, "y1")

        v = nc.vector

        def scalar_chain(ss, am, s_out, c_out, d, dw_col):
            """s_out = 127/absmax; c_out = am*sqrt(d)*dw/(127*max(n,1e-12))."""
            v.tensor_scalar(t8a[:], am[:], 1e-20, None, OP.max)
            v.reciprocal(t8b[:], t8a[:])
            v.tensor_scalar(s_out[:], t8b[:], 127.0, None, OP.mult)
            nc.scalar.sqrt(y0[:], ss[:])
            v.tensor_scalar(y1[:], y0[:], 1e-20, None, OP.max)
            v.reciprocal(t8b[:], y1[:])
            v.tensor_mul(t8a[:], ss[:], t8b[:])            # ss/y
            v.tensor_add(t8c[:], t8a[:], y1[:])            # y + ss/y
            v.tensor_scalar(y1[:], t8c[:], 0.5, 1e-12, OP.mult, OP.max)  # n
            v.reciprocal(t8b[:], y1[:])                    # 1/n
            v.tensor_mul(t8a[:], am[:], t8b[:])            # am/n
            v.tensor_scalar(t8b[:], t8a[:], float(np.sqrt(d) / 127.0), None, OP.mult)
            v.tensor_scalar(c_out[:], t8b[:], dw[:, dw_col:dw_col + 1], None, OP.mult)

        # ---- Phase S: weight mean|.| partial sums over this core's shard ----
        ones = pt([P, P], F32, "ones")
        v.memset(ones[:], 1.0)
        with (
            tc.tile_pool(name="wstat", bufs=3) as wsp,
            tc.tile_pool(name="wstps", bufs=2, space="PSUM") as wsps,
        ):
            def wstats(src_d, rows_total, width, out_col, tmp):
                n_t = (rows_total + P - 1) // P
                n_ch = (width + CW - 1) // CW
                parts = pt([P, n_t * n_ch], F32, f"parts{out_col}")
                v.memset(parts[:], 0.0)
                for i in range(n_t):
                    rows = min(P, rows_total - i * P)
                    for j in range(n_ch):
                        w = min(CW, width - j * CW)
                        wt = wsp.tile([P, CW], F32, tag="ws", name="ws")
                        nc.sync.dma_start(wt[:rows, :w],
                                          src_d[i * P:i * P + rows,
                                                j * CW:j * CW + w])
                        nc.scalar.activation(
                            junk[:rows, :w], wt[:rows, :w], AF.Abs,
                            accum_out=parts[:rows, i * n_ch + j:i * n_ch + j + 1])
                v.tensor_reduce(tmp[:, 0:1], parts[:], axis=AX.X, op=OP.add)
                # cross-partition sum + broadcast in one matmul: ones^T @ tmp
                psb = wsps.tile([P, 1], F32, tag="psb", name="psb")
                nc.tensor.matmul(psb[:], ones[:], tmp[:, 0:1], start=True, stop=True)
                v.tensor_copy(wsum[:, out_col:out_col + 1], psb[:])

            wstats(w1s_d, d_shard, I, 0, t8a)
            wstats(w2s_d, i_shard, D, 1, t8b)

        if not full_stats:
            # combine shard sums across cores: one tiny ([1,2]) AllReduce
            nc.sync.dma_start(cc_in[0:1, :], wsum[0:1, :])
            nc.gpsimd.collective_compute(
                "AllReduce", OP.add,
                replica_groups=[list(range(n_cores))],
                ins=[cc_in.ap().opt()], outs=[cc_out.ap().opt()],
            )
            sc2 = pt([1, 2], F32, "sc2")
            nc.sync.dma_start(sc2[:], cc_out[0:1, :])
            # broadcast partition 0 -> all partitions via a K=1 matmul
            ones1 = pt([1, P], F32, "ones1")
            v.memset(ones1[:], 1.0)
            with tc.tile_pool(name="bcps", bufs=1, space="PSUM") as bcps:
                psb2 = bcps.tile([P, 2], F32, tag="psb2", name="psb2")
                nc.tensor.matmul(psb2[:], ones1[:], sc2[:], start=True, stop=True)
                v.tensor_copy(wsum[:, :], psb2[:])

        # weight dequant factor and quant scale
        v.tensor_scalar(dw[:, 0:1], wsum[:, 0:1], 1.0 / (D * I), EPS, OP.mult, OP.max)
        v.tensor_scalar(dw[:, 1:2], wsum[:, 1:2], 1.0 / (D * I), EPS, OP.mult, OP.max)
        v.reciprocal(sw[:, :], dw[:, :])

        # ---- Phase X: per-token stats, then int-grid quantization of x ----
        # All of xq is staged in one SBUF tensor and written to DRAM by a
        # single DMA: the xbar-transpose loads after it then carry exactly
        # one sem wait (walrus rejects DmaTransposeAnt with >1 wait).
        with tc.tile_pool(name="xph", bufs=2) as xp:
            xq_big = xp.tile([P, n_m * D], BF16, tag="xq_big", name="xq_big", bufs=1)
            xts = []
            for m in range(n_m):
                xt = xp.tile([P, D], F32, tag=f"xt{m}", name=f"xt{m}", bufs=1)
                nc.sync.dma_start(xt[:], x_d[m * P:(m + 1) * P, :])
                nc.scalar.activation(junk[:, :D], xt[:], AF.Square,
                                     accum_out=ss_x[:, m:m + 1])
                v.tensor_reduce(am_x[:, m:m + 1], xt[:], axis=AX.X,
                                op=OP.max, apply_absolute_value=True)
                xts.append(xt)
            scalar_chain(ss_x, am_x, s_x, cm1, D, 0)
            for m in range(n_m):
                tq = xp.tile([P, D], F32, tag="tq", name="tq")
                v.tensor_scalar(tq[:], xts[m][:], s_x[:, m:m + 1], MAGIC,
                                OP.mult, OP.add)
                v.tensor_scalar(xq_big[:, m * D:(m + 1) * D], tq[:], MAGIC, None,
                                OP.subtract, OP.add, accum_out=rs_x[:, m:m + 1])
            nc.sync.dma_start(
                xq_d[:, :].rearrange("(m p) d -> p m d", p=P),
                xq_big[:].rearrange("p (m d) -> p m d", d=D))
            v.tensor_mul(t8c[:], cm1[:], rs_x[:])
            v.tensor_scalar(bias1[:], t8c[:], -WMAGIC, None, OP.mult)

        # ---- Phase XT: transposed xq for the PE contraction ----
        # ---- Phase M1: h = xq @ w1q'^T with fused ternary quant of w1 ----
        with (
            tc.tile_pool(name="xqt", bufs=1) as xqt_pool,
            tc.tile_pool(name="m1w", bufs=3) as m1w,
            tc.tile_pool(name="m1q", bufs=3) as m1q,
            tc.tile_pool(name="m1g", bufs=3) as m1g,
            tc.tile_pool(name="m1ps", bufs=1, space="PSUM") as m1ps,
        ):
            xqt = []
            for k in range(n_k1):
                t = xqt_pool.tile([P, T], BF16, tag=f"xqt{k}", name=f"xqt{k}")
                nc.sync.dma_start(t[:], xq_d[:, k * P:(k + 1) * P], transpose=True)
                xqt.append(t)
            for nb in range(n_nb1):
                ps = [m1ps.tile([P, 512], F32, tag=f"ps{m}", name=f"ps{m}") for m in range(n_m)]
                for k in range(n_k1):
                    wf = m1w.tile([P, 512], F32, tag="wf", name="wf", bufs=6)
                    nc.sync.dma_start(wf[:], w1t_d[k * P:(k + 1) * P,
                                                   nb * 512:(nb + 1) * 512])
                    tf = m1w.tile([P, 512], F32, tag="tf", name="tf")
                    v.tensor_scalar(tf[:], wf[:], sw[:, 0:1], -CLIP_B,
                                    OP.mult, OP.max)
                    tg = m1w.tile([P, 512], F32, tag="tg", name="tg")
                    v.tensor_scalar(tg[:], tf[:], CLIP_B, MAGIC, OP.min, OP.add)
                    wq = m1q.tile([P, 512], BF16, tag="wq", name="wq")
                    v.tensor_scalar(wq[:], tg[:], MAGIC - WMAGIC, None, OP.subtract)
                    for m in range(n_m):
                        nc.tensor.matmul(ps[m][:], xqt[k][:, m * P:(m + 1) * P],
                                         wq[:], start=(k == 0), stop=(k == n_k1 - 1))
                for m in range(n_m):
                    idx = m * n_nb1 + nb
                    g1 = m1g.tile([P, 512], F32, tag="g1", name="g1")
                    nc.scalar.activation(g1[:], ps[m][:], AF.Gelu,
                                         bias=bias1[:, m:m + 1], scale=cm1[:, m:m + 1])
                    g2 = m1g.tile([P, 512], F32, tag="g2", name="g2")
                    nc.scalar.activation(g2[:], g1[:], AF.Gelu)
                    nc.scalar.activation(junk[:, :512], g2[:], AF.Square,
                                         accum_out=ssh_p[:, idx:idx + 1])
                    v.tensor_reduce(amh_p[:, idx:idx + 1], g2[:], axis=AX.X,
                                    op=OP.max, apply_absolute_value=True)
                    nc.sync.dma_start(g2_d[m * P:(m + 1) * P, nb * 512:(nb + 1) * 512],
                                      g2[:])

        # ---- Phase HS: h-layer per-token scales ----
        v.tensor_reduce(ss_h[:], ssh_p[:].rearrange("p (m b) -> p m b", b=n_nb1),
                        axis=AX.X, op=OP.add)
        v.tensor_reduce(am_h[:], amh_p[:].rearrange("p (m b) -> p m b", b=n_nb1),
                        axis=AX.X, op=OP.max)
        scalar_chain(ss_h, am_h, s_h, cm2, I, 1)

        # ---- Phase HQ + HT: quantize g2 -> hq, transpose per column chunk ----
        # Per chunk: all token blocks quantized into one staged SBUF tensor,
        # one funnel DMA to DRAM, then the chunk's xbar transposes (each with
        # a single sem wait on that funnel DMA). The nb=0 weight-quant stream
        # of M2 is interleaved into the chunk loop so wq2 tiles are ready the
        # moment the transposed hq arrives (otherwise the PE idles ~200us at
        # the layer boundary waiting behind the h-quant DMA queue).
        hqt_pool = ctx.enter_context(tc.tile_pool(name="hqt", bufs=1))
        m2w = ctx.enter_context(tc.tile_pool(name="m2w", bufs=3))
        m2q = ctx.enter_context(tc.tile_pool(name="m2q", bufs=3))
        hqt = [None] * n_k2
        wq2_pre = [None] * n_k2
        k_per_c = HQC // P

        def w2_quant(k, nb, bufs=None):
            wf = m2w.tile([P, 512], F32, tag="wf", name="wf", bufs=6)
            nc.sync.dma_start(wf[:], w2t_d[k * P:(k + 1) * P,
                                           nb * 512:(nb + 1) * 512])
            tf = m2w.tile([P, 512], F32, tag="tf", name="tf")
            v.tensor_scalar(tf[:], wf[:], sw[:, 1:2], -CLIP_B, OP.mult, OP.max)
            tg = m2w.tile([P, 512], F32, tag="tg", name="tg")
            v.tensor_scalar(tg[:], tf[:], CLIP_B, MAGIC, OP.min, OP.add)
            wq = m2q.tile([P, 512], BF16, tag="wqp" if bufs else "wq", name="wq",
                          bufs=bufs or 4)
            # exact (magic - 192) subtract on the otherwise-idle ACT engine
            nc.scalar.activation(wq[:], tg[:], AF.Copy,
                                 bias=-(MAGIC - WMAGIC), scale=1.0)
            return wq

        with tc.tile_pool(name="hqp", bufs=2) as hp:
            for c in range(n_hc):
                hq_big = hp.tile([P, n_m * HQC], BF16, tag="hq_big", name="hq_big",
                                 bufs=1)
                for m in range(n_m):
                    gt = hp.tile([P, HQC], F32, tag="gt", name="gt", bufs=4)
                    nc.sync.dma_start(gt[:], g2_d[m * P:(m + 1) * P,
                                                  c * HQC:(c + 1) * HQC])
                    tq = hp.tile([P, HQC], F32, tag="tq", name="tq")
                    nc.gpsimd.tensor_scalar(tq[:], gt[:], s_h[:, m:m + 1], MAGIC,
                                            OP.mult, OP.add)
                    idx = m * n_hc + c
                    v.tensor_scalar(hq_big[:, m * HQC:(m + 1) * HQC], tq[:], MAGIC,
                                    None, OP.subtract, OP.add,
                                    accum_out=rsh_p[:, idx:idx + 1])
                nc.sync.dma_start(
                    hq_d[:, c * HQC:(c + 1) * HQC].rearrange("(m p) d -> p m d", p=P),
                    hq_big[:].rearrange("p (m d) -> p m d", d=HQC))
                for kk in range(k_per_c):
                    k = c * k_per_c + kk
                    t = hqt_pool.tile([P, T], BF16, tag=f"hqt{k}", name=f"hqt{k}")
                    nc.sync.dma_start(t[:], hq_d[:, k * P:(k + 1) * P],
                                      transpose=True)
                    hqt[k] = t
                    wq2_pre[k] = w2_quant(k, 0, bufs=16)
        v.tensor_reduce(rs_h[:], rsh_p[:].rearrange("p (m b) -> p m b", b=n_hc),
                        axis=AX.X, op=OP.add)
        v.tensor_mul(t8a[:], cm2[:], rs_h[:])
        v.tensor_scalar(bias2[:], t8a[:], -WMAGIC, None, OP.mult)

        # ---- Phase M2: out = hq @ w2q'^T with fused ternary quant of w2 ----
        with (
            tc.tile_pool(name="m2o", bufs=3) as m2o,
            tc.tile_pool(name="m2ps", bufs=1, space="PSUM") as m2ps,
        ):
            for nb in range(n_nb2):
                ps = [m2ps.tile([P, 512], F32, tag=f"ps{m}", name=f"ps{m}") for m in range(n_m)]
                for k in range(n_k2):
                    wq = wq2_pre[k] if nb == 0 else w2_quant(k, nb)
                    for m in range(n_m):
                        nc.tensor.matmul(ps[m][:], hqt[k][:, m * P:(m + 1) * P],
                                         wq[:], start=(k == 0), stop=(k == n_k2 - 1))
                for m in range(n_m):
                    o = m2o.tile([P, 512], F32, tag="o", name="o")
                    nc.scalar.activation(o[:], ps[m][:], AF.Identity,
                                         bias=bias2[:, m:m + 1], scale=cm2[:, m:m + 1])
                    nc.sync.dma_start(out_d[m * P:(m + 1) * P, nb * 512:(nb + 1) * 512],
                                      o[:])

    if split_waits:
        _split_waits(nc)
    return nc


# walrus TPB-instruction encodings accept only ONE sem-wait condition on this
# compile path ("Too many sync wait commands"); DMA copies and drains are
# lowered differently and take many. Move extra waits onto standalone
# EventSemaphore (wait_ge-style) instructions just before the instruction on
# the same engine queue -- engine FIFO order preserves semantics exactly.
_WAIT_OK = {"InstEventSemaphore"}


def _split_waits(nc, limit=1):
    cnt = 0
    for fn in nc.m.functions:
        for bb in fn.blocks:
            out = []
            for ins in bb.instructions:
                si = ins.sync_info
                waits = list(si.on_wait) if (si and si.on_wait) else []
                if type(ins).__name__ not in _WAIT_OK and len(waits) > limit:
                    extra, keep = waits[:-limit], waits[-limit:]
                    for w in extra:
                        cnt += 1
                        out.append(mybir.InstEventSemaphore(
                            name=f"WSPLIT-{cnt}-{ins.name}", engine=ins.engine,
                            sync_info=mybir.SyncInfo(on_wait=[w], on_update=[])))
                    try:
                        si.on_wait = keep
                    except Exception:
                        ins.sync_info = mybir.SyncInfo(on_wait=keep,
                                                       on_update=si.on_update)
                out.append(ins)
            bb.instructions[:] = out
    return cnt


_NC_CACHE = {}


def _get_nc(key, **kw):
    if key not in _NC_CACHE:
        _NC_CACHE[key] = build_nc(**kw)
    return _NC_CACHE[key]


def make_in_maps(xf, w1t, w2t, T, n_cores, full_stats=False):
    D, I = w1t.shape
    ds = D if full_stats else D // n_cores
    ish = I if full_stats else I // n_cores
    maps = []
    for c in range(n_cores):
        maps.append({
            "x": np.ascontiguousarray(xf[c * T:(c + 1) * T]),
            "w1t": w1t,
            "w2t": w2t,
            "w1s": w1t if full_stats else np.ascontiguousarray(w1t[c * ds:(c + 1) * ds]),
            "w2s": w2t if full_stats else np.ascontiguousarray(w2t[c * ish:(c + 1) * ish]),
        })
    return maps


def kernel(x, w1, w2):
    from concourse.bass_utils import run_bass_kernel_spmd

    x = np.asarray(x, dtype=np.float32)
    w1 = np.asarray(w1, dtype=np.float32)
    w2 = np.asarray(w2, dtype=np.float32)
    b, s, d = x.shape
    T = (b * s) // N_CORES
    xf = np.ascontiguousarray(x.reshape(b * s, d))
    w1t = np.ascontiguousarray(w1.T)    # [D, I]
    w2t = np.ascontiguousarray(w2.T)    # [I, D]

    nc = _get_nc("main", T=T, D=DIM, I=INNER, n_cores=N_CORES, full_stats=False)
    in_maps = make_in_maps(xf, w1t, w2t, T, N_CORES, full_stats=False)
    res = run_bass_kernel_spmd(nc, in_maps, list(range(N_CORES)))
    outs = [res.results[c]["out"] for c in range(N_CORES)]
    return np.concatenate(outs, axis=0).reshape(b, s, d).astype(np.float32)

